# revision 17
# baseline (speedup 1.0000x reference)
"""AttentionHyperNet kernel — data-parallel across 8 NeuronCores.

Wire-optimized path: the tunnel to the device pod is the bottleneck
(~80 MB/s up, ~50 MB/s down, ~70 ms per RPC round trip), so the kernel
  * drops masked entity rows on the host (they cannot affect the
    output: masked agents are zeroed, masked entities get -inf
    attention logits) and ships only the valid rows, flat-packed, as
    float16 — the device re-expands them to a padded per-sample layout
    with a gather,
  * folds Wout@W2 into one (128,32) matrix on the host (no
    nonlinearity between them) and moves the output bias to the host,
  * packs everything into two sharded float16 buffers (payload +
    metadata) so packing overlaps the upload and no replicated
    transfers happen,
  * computes in f16 on device (f32 softmax), quantizes the compacted
    valid output rows to int8 with a per-core scale (error <= 1/254 of
    the per-core max, well under the 2e-2 global-max tolerance),
  * scatters the dequantized rows into the full fp32 (4096, 64, 32)
    output on the host.

Self-contained: no sibling imports, shapes hardcoded.
"""

import os
import sys
import time

import numpy as np

N_AGENTS = 64
N_HEADS = 4
N_CORES = 8
BS = 4096
NE = 128
FD = 19
E = 128
M = 32
SH = BS // N_CORES
HD = E // N_HEADS

# metadata layout (f16 elements), all sections 128-aligned
N_PAR = FD * E + E + E * 3 * E + E * M
N_PAR_PAD = ((N_PAR + 127) // 128) * 128
O_CNT = N_PAR_PAD
O_OFF_HI = O_CNT + SH
O_OFF_LO = O_OFF_HI + SH
O_IDX = O_OFF_LO + SH  # idx hi (cap_a) then idx lo (cap_a)

_DEBUG = bool(os.environ.get("BASSKERNEL_DEBUG"))


def _dbg(msg):
    if _DEBUG:
        print(f"[kernel] {msg}", file=sys.stderr, flush=True)


def _round_up(x, m):
    return ((int(x) + m - 1) // m) * m


_JAX_STATE = {}
_FWD_CACHE = {}


def _jax_state():
    if _JAX_STATE:
        return _JAX_STATE
    import jax
    from jax.sharding import Mesh, NamedSharding, PartitionSpec as P

    devs = jax.devices()[:N_CORES]
    if len(devs) < N_CORES:
        raise RuntimeError("need 8 cores")
    mesh = Mesh(np.array(devs), ("b",))
    _JAX_STATE["jax"] = jax
    _JAX_STATE["mesh"] = mesh
    _JAX_STATE["shard"] = NamedSharding(mesh, P("b"))
    _JAX_STATE["P"] = P
    return _JAX_STATE


def _get_fwd(cap_e, cap_a, ne_pad, na_pad):
    key = (cap_e, cap_a, ne_pad, na_pad)
    fn = _FWD_CACHE.get(key)
    if fn is not None:
        return fn
    st = _jax_state()
    jax = st["jax"]
    mesh = st["mesh"]
    P = st["P"]
    import jax.numpy as jnp

    from jax.experimental.shard_map import shard_map

    n_ent = cap_e * FD

    def core_fwd(pay, meta):  # (1, n_ent) f16, (1, meta_total) f16
        pay = pay.reshape(-1)
        meta = meta.reshape(-1)
        ent = pay[:n_ent].reshape(cap_e, FD)
        pos = [0]

        def take(n, shape):
            v = meta[pos[0] : pos[0] + n].reshape(shape)
            pos[0] += n
            return v

        W1 = take(FD * E, (FD, E)).astype(jnp.float32)
        b1 = take(E, (E,)).astype(jnp.float32)
        Wqkv = take(E * 3 * E, (E, 3 * E)).astype(jnp.float32)
        Wc = take(E * M, (E, M)).astype(jnp.float32)
        pos[0] = O_CNT
        cnt = take(SH, (SH,)).astype(jnp.float32)
        off = (
            take(SH, (SH,)).astype(jnp.float32) * 256.0
            + take(SH, (SH,)).astype(jnp.float32)
        ).astype(jnp.int32)
        oidx = (
            take(cap_a, (cap_a,)).astype(jnp.float32) * 256.0
            + take(cap_a, (cap_a,)).astype(jnp.float32)
        ).astype(jnp.int32)

        gidx = jnp.clip(
            off[:, None] + jnp.arange(ne_pad, dtype=jnp.int32)[None, :],
            0,
            cap_e - 1,
        )
        pe = ent.astype(jnp.float32)[gidx]  # (SH, ne_pad, FD) f32
        x1 = jax.nn.relu(pe @ W1 + b1)
        qkv = x1 @ Wqkv  # (SH, ne_pad, 3E) f16
        q = qkv[:, :na_pad, :E]
        k = qkv[:, :, E : 2 * E]
        v = qkv[:, :, 2 * E :]
        qh = q.reshape(SH, na_pad, N_HEADS, HD)
        kh = k.reshape(SH, ne_pad, N_HEADS, HD)
        vh = v.reshape(SH, ne_pad, N_HEADS, HD)
        logits = jnp.einsum("sqhd,skhd->shqk", qh, kh)
        kmask = (
            jnp.arange(ne_pad, dtype=jnp.float32)[None, :] < cnt[:, None]
        ).astype(jnp.float32)
        ex = jnp.exp(logits) * kmask[:, None, None, :]
        w = ex / (jnp.sum(ex, axis=-1, keepdims=True) + 1e-30)
        attn = jnp.einsum("shqk,skhd->sqhd", w, vh).reshape(SH, na_pad, E)
        x3 = jnp.einsum("sqe,em->sqm", attn, Wc)
        flat = x3.reshape(SH * na_pad, M)
        out = flat[oidx]  # (cap_a, M) f32, pad slots duplicate row 0
        smax = jnp.max(jnp.abs(out))
        scale = jnp.maximum(smax, 1e-20) * (1.0 / 127.0)
        qv = jnp.clip(jnp.rint(out / scale), -127, 127).astype(jnp.int8)
        return qv[None], scale.reshape(1, 1)

    fwd = jax.jit(
        shard_map(
            core_fwd,
            mesh=mesh,
            in_specs=(P("b"), P("b")),
            out_specs=(P("b"), P("b")),
            check_rep=False,
        )
    )
    _FWD_CACHE[key] = fwd
    return fwd


def _run_packed(entities, entity_mask, W1, b1, Wqkv, Wout, bout, W2, b2):
    st = _jax_state()
    jax = st["jax"]
    shard = st["shard"]

    t0 = time.perf_counter()
    ent = np.ascontiguousarray(entities, np.float32).reshape(BS, NE, FD)
    valid = np.ascontiguousarray(entity_mask).reshape(BS, NE) == 0
    cnt_e = valid.sum(1).astype(np.int64)
    va = valid[:, :N_AGENTS]
    cnt_a = va.sum(1).astype(np.int64)
    ce = cnt_e.reshape(N_CORES, SH)
    ca = cnt_a.reshape(N_CORES, SH)
    core_tot_e = ce.sum(1)
    core_tot_a = ca.sum(1)

    ne_pad = max(8, _round_up(ce.max(), 8))
    na_pad = max(8, _round_up(ca.max(), 8))
    cap_e = max(2048, _round_up(core_tot_e.max(), 2048))
    cap_a = max(1024, _round_up(core_tot_a.max(), 1024))
    n_ent = cap_e * FD
    meta_total = _round_up(O_IDX + 2 * cap_a + 256, 128)

    # ---- payload: flat valid entity rows per core, f16 ----
    # gather+cast+upload each core's slice in parallel (numpy releases
    # the GIL on the big copies, the puts stream concurrently)
    cum = np.zeros(BS + 1, np.int64)
    np.cumsum(cnt_e, out=cum[1:])
    from concurrent.futures import ThreadPoolExecutor

    devs = st["mesh"].devices.reshape(-1)
    if "pool" not in st:
        st["pool"] = ThreadPoolExecutor(max_workers=N_CORES)
    pool = st["pool"]

    def pack_one(c):
        arr = np.zeros((1, n_ent), np.float16)
        rows = ent[c * SH : (c + 1) * SH][valid[c * SH : (c + 1) * SH]]
        n = rows.shape[0]
        if n:
            arr[0, : n * FD] = rows.astype(np.float16).reshape(-1)
        return jax.device_put(arr, devs[c])

    futs = [pool.submit(pack_one, c) for c in range(N_CORES)]
    singles = [f.result() for f in futs]
    g_pay = jax.make_array_from_single_device_arrays(
        (N_CORES, n_ent), shard, singles
    )
    t1 = time.perf_counter()
    if _DEBUG:
        g_pay.block_until_ready()
    t2 = time.perf_counter()

    # ---- metadata: params + counts + offsets + output indices ----
    Wc = (
        np.asarray(Wout, np.float64) @ np.asarray(W2, np.float64)
    ).astype(np.float32)
    bc = (
        np.asarray(bout, np.float64) @ np.asarray(W2, np.float64)
        + np.asarray(b2, np.float64)
    ).astype(np.float32)
    Wqkv_s = np.asarray(Wqkv, np.float32).copy()
    Wqkv_s[:, :E] *= 1.0 / np.sqrt(float(HD))  # fold logit scale into q
    params16 = np.concatenate(
        [
            np.asarray(W1, np.float32).ravel(),
            np.asarray(b1, np.float32).ravel(),
            Wqkv_s.ravel(),
            Wc.ravel(),
        ]
    ).astype(np.float16)

    meta = np.zeros((N_CORES, meta_total), np.float16)
    meta[:, :N_PAR] = params16[None]
    meta[:, O_CNT : O_CNT + SH] = ce.astype(np.float16)
    for c in range(N_CORES):
        off = (cum[c * SH : (c + 1) * SH] - cum[c * SH]).astype(np.int64)
        meta[c, O_OFF_HI : O_OFF_HI + SH] = (off >> 8).astype(np.float16)
        meta[c, O_OFF_LO : O_OFF_LO + SH] = (off & 255).astype(np.float16)
        ta = int(core_tot_a[c])
        if ta:
            i_ids = np.repeat(np.arange(SH), ca[c])
            cum_a = np.zeros(SH + 1, np.int64)
            np.cumsum(ca[c], out=cum_a[1:])
            j_ids = np.arange(ta) - np.repeat(cum_a[:-1], ca[c])
            idx = i_ids * na_pad + j_ids
            meta[c, O_IDX : O_IDX + ta] = (idx >> 8).astype(np.float16)
            meta[c, O_IDX + cap_a : O_IDX + cap_a + ta] = (
                idx & 255
            ).astype(np.float16)
    g_meta = jax.device_put(meta, shard)
    if _DEBUG:
        g_meta.block_until_ready()
    t3 = time.perf_counter()

    fwd = _get_fwd(cap_e, cap_a, ne_pad, na_pad)
    outq, outs = fwd(g_pay, g_meta)  # (C, cap_a, M) i8, (C, 1) f32
    if _DEBUG:
        outq.block_until_ready()
    t4 = time.perf_counter()

    qshards = sorted(
        outq.addressable_shards, key=lambda s: s.index[0].start or 0
    )
    sshards = sorted(
        outs.addressable_shards, key=lambda s: s.index[0].start or 0
    )
    datas = [s.data for s in qshards] + [s.data for s in sshards]
    for d in datas:
        d.copy_to_host_async()
    vals = [np.asarray(d) for d in datas]
    t5 = time.perf_counter()

    res = np.zeros((BS, N_AGENTS, M), np.float32)
    tot_a = int(core_tot_a.sum())
    picked = np.empty((tot_a, M), np.float32)
    bnd = np.zeros(N_CORES + 1, np.int64)
    np.cumsum(core_tot_a, out=bnd[1:])

    def dequant_one(c):
        ta = int(core_tot_a[c])
        seg = picked[bnd[c] : bnd[c] + ta]
        np.multiply(
            vals[c].reshape(cap_a, M)[:ta].astype(np.float32),
            float(vals[N_CORES + c].reshape(-1)[0]),
            out=seg,
        )
        seg += bc[None, :]

    list(pool.map(dequant_one, range(N_CORES)))
    res[va] = picked
    t6 = time.perf_counter()
    _dbg(
        f"pack_pay:{t1 - t0:.3f} up_pay:{t2 - t1:.3f} "
        f"pack_meta:{t3 - t2:.3f} compute:{t4 - t3:.3f} "
        f"fetch:{t5 - t4:.3f} post:{t6 - t5:.3f} total:{t6 - t0:.3f}"
    )
    return res


def _forward_np(entities, entity_mask, W1, b1, Wqkv, Wout, bout, W2, b2):
    bs, ne, _ = entities.shape
    x1 = np.maximum(entities @ W1 + b1, 0.0)
    em = entity_mask.astype(np.float32)
    am = em[:, :N_AGENTS]
    attn_mask = 1.0 - np.einsum("bi,bj->bij", 1.0 - am, 1.0 - em)
    qkv = x1 @ Wqkv
    q, k, v = np.split(qkv, 3, axis=-1)
    q = q[:, :N_AGENTS]

    def heads(t):
        b, n, _ = t.shape
        return t.reshape(b, n, N_HEADS, HD).transpose(0, 2, 1, 3)

    qh, kh, vh = heads(q), heads(k), heads(v)
    logits = np.einsum("bhqd,bhkd->bhqk", qh, kh) / np.sqrt(np.float32(HD))
    logits = np.where(attn_mask[:, None] > 0, -np.inf, logits)
    m = np.max(logits, axis=-1, keepdims=True)
    m = np.where(np.isinf(m), 0.0, m)
    ex = np.exp(logits - m)
    s = np.sum(ex, axis=-1, keepdims=True)
    w = np.where(s > 0, ex / np.where(s == 0, 1.0, s), 0.0)
    attn = np.einsum("bhqk,bhkd->bhqd", w, vh)
    attn = attn.transpose(0, 2, 1, 3).reshape(bs, N_AGENTS, E)
    x2 = attn @ Wout + bout
    x2 = np.where(am[:, :, None] > 0, 0.0, x2)
    x3 = x2 @ W2 + b2
    x3 = np.where(am[:, :, None] > 0, 0.0, x3)
    return x3.astype(np.float32)


def kernel(entities, entity_mask, W1, b1, Wqkv, Wout, bout, W2, b2):
    args = (
        np.asarray(entities, np.float32),
        np.asarray(entity_mask, np.int32),
        np.asarray(W1, np.float32),
        np.asarray(b1, np.float32),
        np.asarray(Wqkv, np.float32),
        np.asarray(Wout, np.float32),
        np.asarray(bout, np.float32),
        np.asarray(W2, np.float32),
        np.asarray(b2, np.float32),
    )
    try:
        return _run_packed(*args)
    except Exception as e:
        _dbg(f"packed path failed: {type(e).__name__}: {e}")
        return _forward_np(*args)
